# revision 27
# baseline (speedup 1.0000x reference)
"""APINet kernel for 8 Trainium2 NeuronCores.

Strategy (all compute on-device, no collectives needed):
  - Host replicates feats (as transposed bf16 hi/lo splits) to all 8 cores;
    each core owns 512 rows (data-parallel over N).
  - Phase 1 (per core): pairwise-distance scores for its 512 rows against
    all 4096 columns via bf16x3 matmuls (hi*hi + hi*lo + lo*hi accumulated
    in f32 PSUM, accuracy ~2.5e-4 on a min-gap of 4e-3), class masks as
    exact additive {0, -BIG} penalties, per-chunk top-2 scan with
    nc.vector.max/max_index, then a tiny cross-chunk argmax merge.
    Yields intra/inter nearest-neighbor indices identical to the f32
    reference argmin/argsort.
  - Phase 2: indirect-DMA row gathers of the neighbor features (bf16) and
    PE transposes into [D, rows] layout.
  - Phase 3: the APINet MLP entirely in [feature, row] layout (bf16
    matmuls, f32 PSUM): h = x@w1x + y@w1y, m = h@w2, sigmoid gates, and
    four fc heads with a shared x@wfc term; outputs sigmoid logits f32.
  - Host reassembles the (8192, 200) logits and integer labels.
"""

import numpy as np
import ml_dtypes

import concourse.bass as bass
import concourse.bacc as bacc
import concourse.mybir as mybir
import concourse.tile as tile
from concourse.bass_utils import run_bass_kernel_spmd

P = 128
N, D, H, C = 4096, 2048, 512, 200
NCORES = 8
R = N // NCORES          # 512 rows per core
RT = R // P              # 4 row-tiles
KD = D // P              # 16 k-tiles over D
KH = H // P              # 4 k-tiles over H
NCHUNK = 8               # column chunks of 512
CW = N // NCHUNK         # 512 chunk width
BIG = 1.0e6

f32 = mybir.dt.float32
bf16 = mybir.dt.bfloat16
i32 = mybir.dt.int32
u32 = mybir.dt.uint32
AX = mybir.AxisListType.X
OP = mybir.AluOpType
AF = mybir.ActivationFunctionType

_BUILT = None


def _phase1(nc, tc, ins, cp, bp):
    """Distance scores + masked per-chunk top-2 + cross-chunk merge.

    Returns (myhi tiles, idx tiles dict, const tiles dict).
    """
    (fallT_hi, fallT_lo, fmyT_hi, fmyT_lo, sqh_b, tall_b, tmy) = ins

    c1p = tc.alloc_tile_pool(name="p1const", bufs=1)
    myhi = [cp.tile([P, R], bf16, tag=f"myhi{k}", name=f"myhi{k}")
            for k in range(KD)]
    mylo = [c1p.tile([P, R], bf16, tag=f"mylo{k}", name=f"mylo{k}")
            for k in range(KD)]
    tallb = c1p.tile([P, N], bf16, tag="tallb")
    sqhb = c1p.tile([P, N], f32, tag="sqhb")
    tmyt = []
    for rt in range(RT):
        t = c1p.tile([P, 1], f32, tag=f"tmy{rt}")
        nc.sync.dma_start(out=t[:], in_=tmy[rt * P:(rt + 1) * P, :])
        tmyt.append(t)

    vbuf = {}
    ibuf = {}
    for rt in range(RT):
        for m in range(2):  # 0 = inter, 1 = intra
            vbuf[rt, m] = bp.tile([P, 2 * NCHUNK], f32, tag=f"v{rt}_{m}", name=f"v{rt}_{m}")
            ibuf[rt, m] = bp.tile([P, 2 * NCHUNK], f32, tag=f"i{rt}_{m}", name=f"i{rt}_{m}")

    sp = tc.alloc_tile_pool(name="p1stream", bufs=3)
    mp = tc.alloc_tile_pool(name="p1mask", bufs=3)
    pp1 = tc.alloc_tile_pool(name="p1psum", bufs=2, space="PSUM")
    for n in range(NCHUNK):
        nsl = slice(n * CW, (n + 1) * CW)
        Pt = [pp1.tile([P, CW], f32, tag=f"P{rt}", name=f"Pt{rt}") for rt in range(RT)]
        if n == 0:
            nc.sync.dma_start(out=sqhb[:, nsl], in_=sqh_b[:, nsl])
            nc.sync.dma_start(out=tallb[:, nsl], in_=tall_b[:, nsl])
        for k in range(KD):
            if n == 0:
                nc.sync.dma_start(out=myhi[k][:],
                                  in_=fmyT_hi[k * P:(k + 1) * P, :])
                nc.sync.dma_start(out=mylo[k][:],
                                  in_=fmyT_lo[k * P:(k + 1) * P, :])
            elif k == 0:
                nc.sync.dma_start(out=sqhb[:, nsl], in_=sqh_b[:, nsl])
                nc.sync.dma_start(out=tallb[:, nsl], in_=tall_b[:, nsl])
            ahi = sp.tile([P, CW], bf16, tag="ahi")
            nc.sync.dma_start(out=ahi[:], in_=fallT_hi[k * P:(k + 1) * P, nsl])
            alo = sp.tile([P, CW], bf16, tag="alo")
            nc.sync.dma_start(out=alo[:], in_=fallT_lo[k * P:(k + 1) * P, nsl])
            for rt in range(RT):
                rsl = slice(rt * P, (rt + 1) * P)
                nc.tensor.matmul(out=Pt[rt][:], lhsT=myhi[k][:, rsl],
                                 rhs=ahi[:], start=(k == 0), stop=False)
                nc.tensor.matmul(out=Pt[rt][:], lhsT=myhi[k][:, rsl],
                                 rhs=alo[:], start=False, stop=False)
                nc.tensor.matmul(out=Pt[rt][:], lhsT=mylo[k][:, rsl],
                                 rhs=ahi[:], start=False, stop=(k == KD - 1))
        for rt in range(RT):
            baset = mp.tile([P, CW], f32, tag="base")
            nc.vector.tensor_tensor(out=baset[:], in0=Pt[rt][:],
                                    in1=sqhb[:, nsl], op=OP.subtract)
            q1 = mp.tile([P, CW], f32, tag="q1")
            nc.vector.tensor_scalar(out=q1[:], in0=tallb[:, nsl],
                                    scalar1=tmyt[rt][:, :1], scalar2=BIG,
                                    op0=OP.is_equal, op1=OP.mult)
            q2 = mp.tile([P, CW], f32, tag="q2")
            nc.vector.tensor_scalar(out=q2[:], in0=tallb[:, nsl],
                                    scalar1=tmyt[rt][:, :1], scalar2=BIG,
                                    op0=OP.not_equal, op1=OP.mult)
            for m, q in ((0, q1), (1, q2)):
                mt = mp.tile([P, CW], f32, tag=f"mt{m}")
                nc.vector.tensor_tensor(out=mt[:], in0=baset[:], in1=q[:],
                                        op=OP.subtract)
                v8 = mp.tile([P, 8], f32, tag="v8")
                nc.vector.max(out=v8[:], in_=mt[:])
                i8 = mp.tile([P, 8], u32, tag="i8")
                nc.vector.max_index(out=i8[:], in_max=v8[:], in_values=mt[:])
                nc.vector.tensor_copy(out=vbuf[rt, m][:, 2 * n:2 * n + 2],
                                      in_=v8[:, :2])
                i2f = mp.tile([P, 2], f32, tag="i2f")
                nc.vector.tensor_copy(out=i2f[:], in_=i8[:, :2])
                nc.vector.tensor_scalar(out=ibuf[rt, m][:, 2 * n:2 * n + 2],
                                        in0=i2f[:], scalar1=float(n * CW),
                                        scalar2=None, op0=OP.add)
    pp1.release()
    mp.release()
    sp.release()
    c1p.release()

    return myhi, vbuf, ibuf


def _merge_one(nc, bp, vbuf, ibuf, rt, m):
    """Cross-chunk merge -> final index tile for one (rt, mask)."""
    if True:
        if True:
            g8 = bp.tile([P, 8], f32, tag=f"g8_{rt}_{m}", name=f"g8_{rt}_{m}")
            nc.vector.max(out=g8[:], in_=vbuf[rt, m][:])
            # inter wants rank 0; intra wants rank 1 (rank 0 is self)
            v = g8[:, m:m + 1]
            eq = bp.tile([P, 2 * NCHUNK], f32, tag=f"eq_{rt}_{m}", name=f"eq_{rt}_{m}")
            nc.vector.tensor_scalar(out=eq[:], in0=vbuf[rt, m][:], scalar1=v,
                                    scalar2=None, op0=OP.is_equal)
            t1 = bp.tile([P, 2 * NCHUNK], f32, tag=f"t1_{rt}_{m}", name=f"t1_{rt}_{m}")
            nc.vector.tensor_tensor(out=t1[:], in0=ibuf[rt, m][:], in1=eq[:],
                                    op=OP.mult)
            t2 = bp.tile([P, 2 * NCHUNK], f32, tag=f"t2_{rt}_{m}", name=f"t2_{rt}_{m}")
            nc.vector.tensor_scalar(out=t2[:], in0=eq[:], scalar1=-1.0e9,
                                    scalar2=1.0e9, op0=OP.mult, op1=OP.add)
            cand = bp.tile([P, 2 * NCHUNK], f32, tag=f"cand_{rt}_{m}", name=f"cand_{rt}_{m}")
            nc.vector.tensor_tensor(out=cand[:], in0=t1[:], in1=t2[:], op=OP.add)
            idxf = bp.tile([P, 1], f32, tag=f"idxf_{rt}_{m}", name=f"idxf_{rt}_{m}")
            nc.vector.tensor_reduce(idxf[:], cand[:], AX, OP.min)
            ii = bp.tile([P, 1], i32, tag=f"idxi_{rt}_{m}", name=f"idxi_{rt}_{m}")
            nc.vector.tensor_copy(out=ii[:], in_=idxf[:])
    return ii


def _build():
    nc = bacc.Bacc(None, target_bir_lowering=False, debug=False)
    dp = nc.declare_dram_parameter

    fallT_hi = dp("fallT_hi", [D, N], bf16, isOutput=False)
    fallT_lo = dp("fallT_lo", [D, N], bf16, isOutput=False)
    fmyT_hi = dp("fmyT_hi", [D, R], bf16, isOutput=False)
    fmyT_lo = dp("fmyT_lo", [D, R], bf16, isOutput=False)
    feats16 = dp("feats16", [N, D], bf16, isOutput=False)
    sqh_b = dp("sqh_b", [P, N], f32, isOutput=False)
    tall_b = dp("tall_b", [P, N], bf16, isOutput=False)
    tmy = dp("tmy", [R, 1], f32, isOutput=False)
    w1x16 = dp("w1x16", [D, H], bf16, isOutput=False)
    w1y16 = dp("w1y16", [D, H], bf16, isOutput=False)
    w2_16 = dp("w2_16", [H, D], bf16, isOutput=False)
    wfc16 = dp("wfc16", [D, C], bf16, isOutput=False)
    b1r = dp("b1r", [1, H], bf16, isOutput=False)
    b2r = dp("b2r", [1, D], bf16, isOutput=False)
    bfcr = dp("bfcr", [1, C], bf16, isOutput=False)
    onesr = dp("onesr", [1, CW], bf16, isOutput=False)
    ident = dp("ident", [P, P], bf16, isOutput=False)
    identf = dp("identf", [P, P], f32, isOutput=False)
    bfcc = dp("bfcc", [C, 1], f32, isOutput=False)

    o_ext = [dp(f"o{j}", [2 * R, C], f32, isOutput=True) for j in range(4)]
    inters_o = dp("inters_o", [R, 1], i32, isOutput=True)
    intras_o = dp("intras_o", [R, 1], i32, isOutput=True)

    with tile.TileContext(nc) as tc:
        with (
            tc.tile_pool(name="const", bufs=1) as cp,
            tc.tile_pool(name="p1buf", bufs=1) as bp,
        ):
            # ---- phase 1: distance scores + per-chunk top-2 scans ----
            myhi, vbuf, ibuf = _phase1(
                nc, tc,
                (fallT_hi, fallT_lo, fmyT_hi, fmyT_lo, sqh_b, tall_b, tmy),
                cp, bp)

            # phase-3 pools that we want alive early (hx/zx0 fill the
            # scan->gather latency gap with y-independent matmuls)
            wp = tc.alloc_tile_pool(name="wstream", bufs=3)
            g3 = tc.alloc_tile_pool(name="p3work", bufs=2)
            gw = tc.alloc_tile_pool(name="gatework", bufs=3)
            accp = tc.alloc_tile_pool(name="accpsum", bufs=2, space="PSUM")

            onest = cp.tile([1, CW], bf16, tag="ones")
            nc.sync.dma_start(out=onest[:], in_=onesr[:])
            b1t = cp.tile([1, H], bf16, tag="b1")
            nc.sync.dma_start(out=b1t[:], in_=b1r[:])
            wfct = []
            for k in range(KD):
                t = cp.tile([P, C], bf16, tag=f"wfc{k}")
                nc.sync.dma_start(out=t[:], in_=wfc16[k * P:(k + 1) * P, :])
                wfct.append(t)

            # 3a: hx = x @ w1x + b1 (shared by both halves)
            hxsb = []
            for h in range(KH):
                hsl = slice(h * P, (h + 1) * P)
                px = accp.tile([P, R], f32, tag="acc")
                for k in range(KD):
                    w = wp.tile([P, P], bf16, tag="w1x")
                    nc.sync.dma_start(out=w[:], in_=w1x16[k * P:(k + 1) * P, hsl])
                    nc.tensor.matmul(out=px[:], lhsT=w[:], rhs=myhi[k][:],
                                     start=(k == 0), stop=False)
                nc.tensor.matmul(out=px[:], lhsT=b1t[:1, hsl], rhs=onest[:1, :R],
                                 start=False, stop=True)
                hx = cp.tile([P, R], bf16, tag=f"hx{h}")
                nc.scalar.activation(hx[:], px[:], AF.Copy)
                hxsb.append(hx)

            bfcct = []
            for mt, (a, b) in enumerate(((0, P), (P, C))):
                t = cp.tile([b - a, 1], f32, tag=f"bfcc{mt}")
                nc.sync.dma_start(out=t[:], in_=bfcc[a:b, :])
                bfcct.append(t)
            identft = cp.tile([P, P], f32, tag="identf")
            nc.sync.dma_start(out=identft[:], in_=identf[:])

            # ---- phase 2: gather neighbor rows (bf16) + transpose ----
            identt = cp.tile([P, P], bf16, tag="ident")
            nc.sync.dma_start(out=identt[:], in_=ident[:])
            yT16 = cp.tile([P, KD, 2 * R], bf16, tag="yT16")
            gp = tc.alloc_tile_pool(name="gpool", bufs=3)
            tpp = tc.alloc_tile_pool(name="tpsum", bufs=2, space="PSUM")

            # ---- phase 3 (rest): needs y ----
            b2t = cp.tile([1, D], bf16, tag="b2")
            nc.sync.dma_start(out=b2t[:], in_=b2r[:])
            w2t = []
            for kh in range(KH):
                t = cp.tile([P, D], bf16, tag=f"w2_{kh}")
                nc.sync.dma_start(out=t[:], in_=w2_16[kh * P:(kh + 1) * P, :])
                w2t.append(t)
            hsb = [cp.tile([P, 2 * R], bf16, tag=f"h{kh}", name=f"hsb{kh}")
                   for kh in range(KH)]

            # phase 1b + 2: merge -> gather -> transpose per (rt, m)
            idxt = {}
            for half, m in ((0, 1), (1, 0)):  # half 0 = intra, 1 = inter
                for rt in range(RT):
                    idxt[rt, m] = _merge_one(nc, bp, vbuf, ibuf, rt, m)
                    yg = gp.tile([P, D], bf16, tag="yg")
                    nc.gpsimd.indirect_dma_start(
                        out=yg[:], out_offset=None, in_=feats16[:],
                        in_offset=bass.IndirectOffsetOnAxis(
                            ap=idxt[rt, m][:, :1], axis=0))
                    for d4 in range(KD // 4):
                        pt = tpp.tile([P, 4, P], bf16, tag="tp")
                        for i in range(4):
                            d = d4 * 4 + i
                            nc.tensor.transpose(out=pt[:, i, :],
                                                in_=yg[:, d * P:(d + 1) * P],
                                                identity=identt[:])
                        ccsl = slice(half * R + rt * P, half * R + (rt + 1) * P)
                        nc.any.tensor_copy(out=yT16[:, d4 * 4:(d4 + 1) * 4, ccsl],
                                           in_=pt[:])
            for rt in range(RT):
                nc.sync.dma_start(out=inters_o[rt * P:(rt + 1) * P, :],
                                  in_=idxt[rt, 0][:])
                nc.sync.dma_start(out=intras_o[rt * P:(rt + 1) * P, :],
                                  in_=idxt[rt, 1][:])
            tpp.release()
            gp.release()
            zp = tc.alloc_tile_pool(name="zpsum", bufs=1, space="PSUM")
            tpo = tc.alloc_tile_pool(name="tpopsum", bufs=2, space="PSUM")

            # 3a': h = hx + y @ w1y per half
            for half in range(2):
                csl = slice(half * R, (half + 1) * R)
                for h in range(KH):
                    hsl = slice(h * P, (h + 1) * P)
                    py = accp.tile([P, R], f32, tag="acc")
                    for k in range(KD):
                        w = wp.tile([P, P], bf16, tag="w1y")
                        nc.sync.dma_start(out=w[:],
                                          in_=w1y16[k * P:(k + 1) * P, hsl])
                        nc.tensor.matmul(out=py[:], lhsT=w[:],
                                         rhs=yT16[:, k, csl],
                                         start=(k == 0), stop=(k == KD - 1))
                    nc.vector.tensor_tensor(out=hsb[h][:, csl], in0=py[:],
                                            in1=hxsb[h][:], op=OP.add)

            # 3b+3c fused: per d: m, gates (1+sigmoid stored), and the
            # pair-A fc matmuls software-pipelined one d behind; pair B as a
            # dense tail loop. fc in [C, rows] layout, transposed back at end.
            MT = (slice(0, P), slice(P, C))
            for half in range(2):
                csl = slice(half * R, (half + 1) * R)
                gx1 = {}
                gy1 = {}
                ua = {}
                ub = {}
                zbA = {}
                for mt in range(2):
                    zbA[0, mt] = zp.tile([P if mt == 0 else C - P, R], f32,
                                         tag=f"zb0_{mt}", name=f"zbA0_{mt}")
                    zbA[1, mt] = zp.tile([P if mt == 0 else C - P, R], f32,
                                         tag=f"zb1_{mt}", name=f"zbA1_{mt}")

                def emit_fcA(d):
                    for mt in range(2):
                        wslice = wfct[d][:, MT[mt]]
                        nc.tensor.matmul(out=zbA[0, mt][:], lhsT=wslice,
                                         rhs=ua[d][:], start=(d == 0),
                                         stop=(d == KD - 1))
                        nc.tensor.matmul(out=zbA[1, mt][:], lhsT=wslice,
                                         rhs=ub[d][:], start=(d == 0),
                                         stop=(d == KD - 1))

                for d in range(KD):
                    pm = accp.tile([P, R], f32, tag="acc")
                    dsl = slice(d * P, (d + 1) * P)
                    for kh in range(KH):
                        nc.tensor.matmul(out=pm[:], lhsT=w2t[kh][:, dsl],
                                         rhs=hsb[kh][:, csl],
                                         start=(kh == 0), stop=False)
                    nc.tensor.matmul(out=pm[:], lhsT=b2t[:1, dsl],
                                     rhs=onest[:1, :R], start=False, stop=True)
                    msb = gw.tile([P, R], bf16, tag="msb")
                    nc.scalar.activation(msb[:], pm[:], AF.Copy)
                    p1 = gw.tile([P, R], bf16, tag="p1")
                    nc.vector.tensor_tensor(out=p1[:], in0=msb[:], in1=myhi[d][:],
                                            op=OP.mult)
                    p2 = gw.tile([P, R], bf16, tag="p2")
                    nc.vector.tensor_tensor(out=p2[:], in0=msb[:],
                                            in1=yT16[:, d, csl], op=OP.mult)
                    sx = gw.tile([P, R], bf16, tag="sx")
                    nc.scalar.activation(sx[:], p1[:], AF.Sigmoid)
                    sy = gw.tile([P, R], bf16, tag="sy")
                    nc.scalar.activation(sy[:], p2[:], AF.Sigmoid)
                    g1 = cp.tile([P, R], bf16, tag=f"gx1_{d}", name=f"gx1_{d}")
                    nc.gpsimd.tensor_scalar(out=g1[:], in0=sx[:], scalar1=1.0,
                                            scalar2=None, op0=OP.add)
                    gx1[d] = g1
                    g2 = cp.tile([P, R], bf16, tag=f"gy1_{d}", name=f"gy1_{d}")
                    nc.gpsimd.tensor_scalar(out=g2[:], in0=sy[:], scalar1=1.0,
                                            scalar2=None, op0=OP.add)
                    gy1[d] = g2
                    # pair-A products: x*(1+gx), x*(1+gy)
                    t = gw.tile([P, R], bf16, tag=f"ua{d % 3}", name=f"ua_{d}")
                    nc.vector.tensor_tensor(out=t[:], in0=myhi[d][:], in1=g1[:],
                                            op=OP.mult)
                    ua[d] = t
                    t = gw.tile([P, R], bf16, tag=f"ub{d % 3}", name=f"ub_{d}")
                    nc.vector.tensor_tensor(out=t[:], in0=myhi[d][:], in1=g2[:],
                                            op=OP.mult)
                    ub[d] = t
                    if d > 0:
                        emit_fcA(d - 1)
                emit_fcA(KD - 1)

                def epilogue(zb, j0):
                    for jj in range(2):
                        j = j0 + jj
                        zs = {}
                        for mt in range(2):
                            cn = P if mt == 0 else C - P
                            zt = g3.tile([cn, R], f32, tag=f"zs{mt}",
                                         name=f"zs{mt}")
                            nc.scalar.activation(zt[:], zb[jj, mt][:],
                                                 AF.Sigmoid,
                                                 bias=bfcct[mt][:, :1])
                            zs[mt] = zt
                        for rb in range(RT):
                            rbs = slice(rb * P, (rb + 1) * P)
                            po = tpo.tile([P, C], f32, tag="tpo")
                            nc.tensor.transpose(out=po[:, :P],
                                                in_=zs[0][:, rbs],
                                                identity=identft[:])
                            nc.tensor.transpose(out=po[:, P:C],
                                                in_=zs[1][:, rbs],
                                                identity=identft[:C - P, :C - P])
                            ot = g3.tile([P, C], f32, tag="ot")
                            nc.scalar.activation(ot[:], po[:], AF.Copy)
                            nc.sync.dma_start(
                                out=o_ext[j][half * R + rb * P:
                                             half * R + (rb + 1) * P, :],
                                in_=ot[:])

                epilogue(zbA, 0)

                # pair B: y*(1+gy) -> logit2_self, y*(1+gx) -> logit2_other
                zbB = {}
                for mt in range(2):
                    zbB[0, mt] = zp.tile([P if mt == 0 else C - P, R], f32,
                                         tag=f"zb0_{mt}", name=f"zbB0_{mt}")
                    zbB[1, mt] = zp.tile([P if mt == 0 else C - P, R], f32,
                                         tag=f"zb1_{mt}", name=f"zbB1_{mt}")
                for d in range(KD):
                    t1 = gw.tile([P, R], bf16, tag=f"ua{d % 3}", name=f"va_{d}")
                    nc.vector.tensor_tensor(out=t1[:], in0=yT16[:, d, csl],
                                            in1=gy1[d][:], op=OP.mult)
                    t2 = gw.tile([P, R], bf16, tag=f"ub{d % 3}", name=f"vb_{d}")
                    nc.vector.tensor_tensor(out=t2[:], in0=yT16[:, d, csl],
                                            in1=gx1[d][:], op=OP.mult)
                    for mt in range(2):
                        wslice = wfct[d][:, MT[mt]]
                        nc.tensor.matmul(out=zbB[0, mt][:], lhsT=wslice,
                                         rhs=t1[:], start=(d == 0),
                                         stop=(d == KD - 1))
                        nc.tensor.matmul(out=zbB[1, mt][:], lhsT=wslice,
                                         rhs=t2[:], start=(d == 0),
                                         stop=(d == KD - 1))
                epilogue(zbB, 2)

            tpo.release()
            zp.release()
            accp.release()
            gw.release()
            g3.release()
            wp.release()

    nc.finalize()
    return nc


def _get_built():
    global _BUILT
    if _BUILT is None:
        _BUILT = _build()
    return _BUILT


def kernel(**inputs):
    bf = ml_dtypes.bfloat16
    feats = np.asarray(inputs["feats"], dtype=np.float32)
    targets = np.asarray(inputs["targets"])
    w1 = np.asarray(inputs["w1"], dtype=np.float32)
    b1 = np.asarray(inputs["b1"], dtype=np.float32)
    w2 = np.asarray(inputs["w2"], dtype=np.float32)
    b2 = np.asarray(inputs["b2"], dtype=np.float32)
    wfc = np.asarray(inputs["wfc"], dtype=np.float32)
    bfc = np.asarray(inputs["bfc"], dtype=np.float32)

    f64 = feats.astype(np.float64)
    sqh = (0.5 * (f64 * f64).sum(axis=1)).astype(np.float32)
    featsT = np.ascontiguousarray(feats.T)                # (D, N) f32
    hiT = featsT.astype(bf)
    loT = (featsT - hiT.astype(np.float32)).astype(bf)
    feats16 = np.ascontiguousarray(hiT.T)                 # (N, D) bf16
    tf = targets.astype(np.float32)
    tall_b = np.ascontiguousarray(np.broadcast_to(tf.astype(bf), (P, N)))
    sqh_b = np.ascontiguousarray(np.broadcast_to(sqh, (P, N)))

    shared = dict(
        fallT_hi=np.ascontiguousarray(hiT),
        fallT_lo=np.ascontiguousarray(loT),
        feats16=feats16,
        sqh_b=sqh_b,
        tall_b=tall_b,
        w1x16=w1[:D].astype(bf),
        w1y16=w1[D:].astype(bf),
        w2_16=w2.astype(bf),
        wfc16=wfc.astype(bf),
        b1r=b1.reshape(1, H).astype(bf),
        b2r=b2.reshape(1, D).astype(bf),
        bfcr=bfc.reshape(1, C).astype(bf),
        onesr=np.ones((1, CW), dtype=bf),
        ident=np.eye(P, dtype=np.float32).astype(bf),
        identf=np.eye(P, dtype=np.float32),
        bfcc=bfc.reshape(C, 1).astype(np.float32),
    )
    in_maps = []
    for c in range(NCORES):
        rs = slice(c * R, (c + 1) * R)
        m = dict(shared)
        m["fmyT_hi"] = np.ascontiguousarray(hiT[:, rs])
        m["fmyT_lo"] = np.ascontiguousarray(loT[:, rs])
        m["tmy"] = np.ascontiguousarray(tf[rs].reshape(R, 1))
        in_maps.append(m)

    nc = _get_built()
    res = run_bass_kernel_spmd(nc, in_maps, core_ids=list(range(NCORES)),
                               trace=False)
    rs_ = res.results

    o = [np.empty((2 * N, C), dtype=np.float32) for _ in range(4)]
    inters_all = np.empty(N, dtype=np.int64)
    intras_all = np.empty(N, dtype=np.int64)
    for c in range(NCORES):
        for j in range(4):
            blk = rs_[c][f"o{j}"]
            o[j][c * R:(c + 1) * R] = blk[:R]
            o[j][N + c * R:N + (c + 1) * R] = blk[R:]
        inters_all[c * R:(c + 1) * R] = rs_[c]["inters_o"][:, 0]
        intras_all[c * R:(c + 1) * R] = rs_[c]["intras_o"][:, 0]

    kernel.last_idx = (intras_all, inters_all)
    labels1 = np.concatenate([targets, targets])
    labels2 = np.concatenate([targets, targets[inters_all]])
    return (o[0], o[1], o[2], o[3], labels1, labels2)


# revision 28
# speedup vs baseline: 1.6117x; 1.6117x over previous
"""APINet kernel for 8 Trainium2 NeuronCores.

Strategy (all compute on-device, no collectives needed):
  - Host replicates feats (as transposed bf16 hi/lo splits) to all 8 cores;
    each core owns 512 rows (data-parallel over N).
  - Phase 1 (per core): pairwise-distance scores for its 512 rows against
    all 4096 columns via bf16x3 matmuls (hi*hi + hi*lo + lo*hi accumulated
    in f32 PSUM, accuracy ~2.5e-4 on a min-gap of 4e-3), class masks as
    exact additive {0, -BIG} penalties, per-chunk top-2 scan with
    nc.vector.max/max_index, then a tiny cross-chunk argmax merge.
    Yields intra/inter nearest-neighbor indices identical to the f32
    reference argmin/argsort.
  - Phase 2: indirect-DMA row gathers of the neighbor features (bf16) and
    PE transposes into [D, rows] layout.
  - Phase 3: the APINet MLP entirely in [feature, row] layout (bf16
    matmuls, f32 PSUM): h = x@w1x + y@w1y, m = h@w2, sigmoid gates, and
    four fc heads with a shared x@wfc term; outputs sigmoid logits f32.
  - Host reassembles the (8192, 200) logits and integer labels.
"""

import numpy as np
import ml_dtypes

import concourse.bass as bass
import concourse.bacc as bacc
import concourse.mybir as mybir
import concourse.tile as tile
from concourse.bass_utils import run_bass_kernel_spmd

P = 128
N, D, H, C = 4096, 2048, 512, 200
NCORES = 8
R = N // NCORES          # 512 rows per core
RT = R // P              # 4 row-tiles
KD = D // P              # 16 k-tiles over D
KH = H // P              # 4 k-tiles over H
NCHUNK = 8               # column chunks of 512
CW = N // NCHUNK         # 512 chunk width
BIG = 1.0e6

f32 = mybir.dt.float32
bf16 = mybir.dt.bfloat16
i32 = mybir.dt.int32
u32 = mybir.dt.uint32
AX = mybir.AxisListType.X
OP = mybir.AluOpType
AF = mybir.ActivationFunctionType

_BUILT = None


def _phase1(nc, tc, ins, cp, bp):
    """Distance scores + masked per-chunk top-2 + cross-chunk merge.

    Returns (myhi tiles, idx tiles dict, const tiles dict).
    """
    (fallT_hi, fallT_lo, fmyT_hi, fmyT_lo, sqh_b, tall_b, tmy) = ins

    c1p = tc.alloc_tile_pool(name="p1const", bufs=1)
    myhi = [cp.tile([P, R], bf16, tag=f"myhi{k}", name=f"myhi{k}")
            for k in range(KD)]
    mylo = [c1p.tile([P, R], bf16, tag=f"mylo{k}", name=f"mylo{k}")
            for k in range(KD)]
    tallb = c1p.tile([P, N], bf16, tag="tallb")
    sqhb = c1p.tile([P, N], f32, tag="sqhb")
    tmyt = []
    for rt in range(RT):
        t = c1p.tile([P, 1], f32, tag=f"tmy{rt}")
        nc.sync.dma_start(out=t[:], in_=tmy[rt * P:(rt + 1) * P, :])
        tmyt.append(t)

    vbuf = {}
    ibuf = {}
    for rt in range(RT):
        for m in range(2):  # 0 = inter, 1 = intra
            vbuf[rt, m] = bp.tile([P, 2 * NCHUNK], f32, tag=f"v{rt}_{m}", name=f"v{rt}_{m}")
            ibuf[rt, m] = bp.tile([P, 2 * NCHUNK], f32, tag=f"i{rt}_{m}", name=f"i{rt}_{m}")

    sp = tc.alloc_tile_pool(name="p1stream", bufs=3)
    mp = tc.alloc_tile_pool(name="p1mask", bufs=3)
    pp1 = tc.alloc_tile_pool(name="p1psum", bufs=2, space="PSUM")
    for n in range(NCHUNK):
        nsl = slice(n * CW, (n + 1) * CW)
        Pt = [pp1.tile([P, CW], f32, tag=f"P{rt}", name=f"Pt{rt}") for rt in range(RT)]
        if n == 0:
            nc.sync.dma_start(out=sqhb[:, nsl], in_=sqh_b[:, nsl])
            nc.sync.dma_start(out=tallb[:, nsl], in_=tall_b[:, nsl])
        for k in range(KD):
            if n == 0:
                nc.sync.dma_start(out=myhi[k][:],
                                  in_=fmyT_hi[k * P:(k + 1) * P, :])
                nc.sync.dma_start(out=mylo[k][:],
                                  in_=fmyT_lo[k * P:(k + 1) * P, :])
            elif k == 0:
                nc.sync.dma_start(out=sqhb[:, nsl], in_=sqh_b[:, nsl])
                nc.sync.dma_start(out=tallb[:, nsl], in_=tall_b[:, nsl])
            ahi = sp.tile([P, CW], bf16, tag="ahi")
            nc.sync.dma_start(out=ahi[:], in_=fallT_hi[k * P:(k + 1) * P, nsl])
            alo = sp.tile([P, CW], bf16, tag="alo")
            nc.sync.dma_start(out=alo[:], in_=fallT_lo[k * P:(k + 1) * P, nsl])
            for rt in range(RT):
                rsl = slice(rt * P, (rt + 1) * P)
                nc.tensor.matmul(out=Pt[rt][:], lhsT=myhi[k][:, rsl],
                                 rhs=ahi[:], start=(k == 0), stop=False)
                nc.tensor.matmul(out=Pt[rt][:], lhsT=myhi[k][:, rsl],
                                 rhs=alo[:], start=False, stop=False)
                nc.tensor.matmul(out=Pt[rt][:], lhsT=mylo[k][:, rsl],
                                 rhs=ahi[:], start=False, stop=(k == KD - 1))
        for rt in range(RT):
            baset = mp.tile([P, CW], f32, tag="base")
            nc.vector.tensor_tensor(out=baset[:], in0=Pt[rt][:],
                                    in1=sqhb[:, nsl], op=OP.subtract)
            q1 = mp.tile([P, CW], f32, tag="q1")
            nc.vector.tensor_scalar(out=q1[:], in0=tallb[:, nsl],
                                    scalar1=tmyt[rt][:, :1], scalar2=BIG,
                                    op0=OP.is_equal, op1=OP.mult)
            q2 = mp.tile([P, CW], f32, tag="q2")
            nc.vector.tensor_scalar(out=q2[:], in0=tallb[:, nsl],
                                    scalar1=tmyt[rt][:, :1], scalar2=BIG,
                                    op0=OP.not_equal, op1=OP.mult)
            for m, q in ((0, q1), (1, q2)):
                mt = mp.tile([P, CW], f32, tag=f"mt{m}")
                nc.vector.tensor_tensor(out=mt[:], in0=baset[:], in1=q[:],
                                        op=OP.subtract)
                v8 = mp.tile([P, 8], f32, tag="v8")
                nc.vector.max(out=v8[:], in_=mt[:])
                i8 = mp.tile([P, 8], u32, tag="i8")
                nc.vector.max_index(out=i8[:], in_max=v8[:], in_values=mt[:])
                nc.vector.tensor_copy(out=vbuf[rt, m][:, 2 * n:2 * n + 2],
                                      in_=v8[:, :2])
                i2f = mp.tile([P, 2], f32, tag="i2f")
                nc.vector.tensor_copy(out=i2f[:], in_=i8[:, :2])
                nc.vector.tensor_scalar(out=ibuf[rt, m][:, 2 * n:2 * n + 2],
                                        in0=i2f[:], scalar1=float(n * CW),
                                        scalar2=None, op0=OP.add)
    pp1.release()
    mp.release()
    sp.release()
    c1p.release()

    return myhi, vbuf, ibuf


def _merge_one(nc, bp, vbuf, ibuf, rt, m):
    """Cross-chunk merge -> final index tile for one (rt, mask)."""
    if True:
        if True:
            g8 = bp.tile([P, 8], f32, tag=f"g8_{rt}_{m}", name=f"g8_{rt}_{m}")
            nc.vector.max(out=g8[:], in_=vbuf[rt, m][:])
            # inter wants rank 0; intra wants rank 1 (rank 0 is self)
            v = g8[:, m:m + 1]
            eq = bp.tile([P, 2 * NCHUNK], f32, tag=f"eq_{rt}_{m}", name=f"eq_{rt}_{m}")
            nc.vector.tensor_scalar(out=eq[:], in0=vbuf[rt, m][:], scalar1=v,
                                    scalar2=None, op0=OP.is_equal)
            t1 = bp.tile([P, 2 * NCHUNK], f32, tag=f"t1_{rt}_{m}", name=f"t1_{rt}_{m}")
            nc.vector.tensor_tensor(out=t1[:], in0=ibuf[rt, m][:], in1=eq[:],
                                    op=OP.mult)
            t2 = bp.tile([P, 2 * NCHUNK], f32, tag=f"t2_{rt}_{m}", name=f"t2_{rt}_{m}")
            nc.vector.tensor_scalar(out=t2[:], in0=eq[:], scalar1=-1.0e9,
                                    scalar2=1.0e9, op0=OP.mult, op1=OP.add)
            cand = bp.tile([P, 2 * NCHUNK], f32, tag=f"cand_{rt}_{m}", name=f"cand_{rt}_{m}")
            nc.vector.tensor_tensor(out=cand[:], in0=t1[:], in1=t2[:], op=OP.add)
            idxf = bp.tile([P, 1], f32, tag=f"idxf_{rt}_{m}", name=f"idxf_{rt}_{m}")
            nc.vector.tensor_reduce(idxf[:], cand[:], AX, OP.min)
            ii = bp.tile([P, 1], i32, tag=f"idxi_{rt}_{m}", name=f"idxi_{rt}_{m}")
            nc.vector.tensor_copy(out=ii[:], in_=idxf[:])
    return ii


def _build():
    nc = bacc.Bacc(None, target_bir_lowering=False, debug=False)
    dp = nc.declare_dram_parameter

    fallT_hi = dp("fallT_hi", [D, N], bf16, isOutput=False)
    fallT_lo = dp("fallT_lo", [D, N], bf16, isOutput=False)
    fmyT_hi = dp("fmyT_hi", [D, R], bf16, isOutput=False)
    fmyT_lo = dp("fmyT_lo", [D, R], bf16, isOutput=False)
    feats16 = dp("feats16", [N, D], bf16, isOutput=False)
    sqh_b = dp("sqh_b", [P, N], f32, isOutput=False)
    tall_b = dp("tall_b", [P, N], bf16, isOutput=False)
    tmy = dp("tmy", [R, 1], f32, isOutput=False)
    w1x16 = dp("w1x16", [D, H], bf16, isOutput=False)
    w1y16 = dp("w1y16", [D, H], bf16, isOutput=False)
    w2_16 = dp("w2_16", [H, D], bf16, isOutput=False)
    wfc16 = dp("wfc16", [D, C], bf16, isOutput=False)
    b1r = dp("b1r", [1, H], bf16, isOutput=False)
    b2r = dp("b2r", [1, D], bf16, isOutput=False)
    bfcr = dp("bfcr", [1, C], bf16, isOutput=False)
    onesr = dp("onesr", [1, CW], bf16, isOutput=False)
    ident = dp("ident", [P, P], bf16, isOutput=False)
    identf = dp("identf", [P, P], f32, isOutput=False)
    bfcc = dp("bfcc", [C, 1], f32, isOutput=False)

    o_ext = [dp(f"o{j}", [2 * R, C], f32, isOutput=True) for j in range(4)]
    inters_o = dp("inters_o", [R, 1], i32, isOutput=True)
    intras_o = dp("intras_o", [R, 1], i32, isOutput=True)

    with tile.TileContext(nc) as tc:
        with (
            tc.tile_pool(name="const", bufs=1) as cp,
            tc.tile_pool(name="p1buf", bufs=1) as bp,
        ):
            # ---- phase 1: distance scores + per-chunk top-2 scans ----
            myhi, vbuf, ibuf = _phase1(
                nc, tc,
                (fallT_hi, fallT_lo, fmyT_hi, fmyT_lo, sqh_b, tall_b, tmy),
                cp, bp)

            # phase-3 pools that we want alive early (hx/zx0 fill the
            # scan->gather latency gap with y-independent matmuls)
            wp = tc.alloc_tile_pool(name="wstream", bufs=3)
            g3 = tc.alloc_tile_pool(name="p3work", bufs=2)
            gw = tc.alloc_tile_pool(name="gatework", bufs=3)
            accp = tc.alloc_tile_pool(name="accpsum", bufs=2, space="PSUM")

            onest = cp.tile([1, CW], bf16, tag="ones")
            nc.sync.dma_start(out=onest[:], in_=onesr[:])
            b1t = cp.tile([1, H], bf16, tag="b1")
            nc.sync.dma_start(out=b1t[:], in_=b1r[:])
            wfct = []
            for k in range(KD):
                t = cp.tile([P, C], bf16, tag=f"wfc{k}")
                nc.sync.dma_start(out=t[:], in_=wfc16[k * P:(k + 1) * P, :])
                wfct.append(t)

            # 3a: hx = x @ w1x + b1 (shared by both halves)
            hxsb = []
            for h in range(KH):
                hsl = slice(h * P, (h + 1) * P)
                px = accp.tile([P, R], f32, tag="acc")
                for k in range(KD):
                    w = wp.tile([P, P], bf16, tag="w1x")
                    nc.sync.dma_start(out=w[:], in_=w1x16[k * P:(k + 1) * P, hsl])
                    nc.tensor.matmul(out=px[:], lhsT=w[:], rhs=myhi[k][:],
                                     start=(k == 0), stop=False)
                nc.tensor.matmul(out=px[:], lhsT=b1t[:1, hsl], rhs=onest[:1, :R],
                                 start=False, stop=True)
                hx = cp.tile([P, R], bf16, tag=f"hx{h}")
                nc.scalar.activation(hx[:], px[:], AF.Copy)
                hxsb.append(hx)

            bfcct = []
            for mt, (a, b) in enumerate(((0, P), (P, C))):
                t = cp.tile([b - a, 1], f32, tag=f"bfcc{mt}")
                nc.sync.dma_start(out=t[:], in_=bfcc[a:b, :])
                bfcct.append(t)
            identft = cp.tile([P, P], f32, tag="identf")
            nc.sync.dma_start(out=identft[:], in_=identf[:])

            # ---- phase 2: gather neighbor rows (bf16) + transpose ----
            identt = cp.tile([P, P], bf16, tag="ident")
            nc.sync.dma_start(out=identt[:], in_=ident[:])
            yT16 = cp.tile([P, KD, 2 * R], bf16, tag="yT16")
            gp = tc.alloc_tile_pool(name="gpool", bufs=3)
            tpp = tc.alloc_tile_pool(name="tpsum", bufs=2, space="PSUM")

            # ---- phase 3 (rest): needs y ----
            b2t = cp.tile([1, D], bf16, tag="b2")
            nc.sync.dma_start(out=b2t[:], in_=b2r[:])
            w2t = []
            for kh in range(KH):
                t = cp.tile([P, D], bf16, tag=f"w2_{kh}")
                nc.sync.dma_start(out=t[:], in_=w2_16[kh * P:(kh + 1) * P, :])
                w2t.append(t)
            hsb = [cp.tile([P, 2 * R], bf16, tag=f"h{kh}", name=f"hsb{kh}")
                   for kh in range(KH)]

            # phase 1b + 2: merge -> gather -> transpose per (rt, m)
            idxt = {}
            for half, m in ((0, 1), (1, 0)):  # half 0 = intra, 1 = inter
                for rt in range(RT):
                    idxt[rt, m] = _merge_one(nc, bp, vbuf, ibuf, rt, m)
                    yg = gp.tile([P, D], bf16, tag="yg")
                    nc.gpsimd.indirect_dma_start(
                        out=yg[:], out_offset=None, in_=feats16[:],
                        in_offset=bass.IndirectOffsetOnAxis(
                            ap=idxt[rt, m][:, :1], axis=0))
                    for d4 in range(KD // 4):
                        pt = tpp.tile([P, 4, P], bf16, tag="tp")
                        for i in range(4):
                            d = d4 * 4 + i
                            nc.tensor.transpose(out=pt[:, i, :],
                                                in_=yg[:, d * P:(d + 1) * P],
                                                identity=identt[:])
                        ccsl = slice(half * R + rt * P, half * R + (rt + 1) * P)
                        nc.any.tensor_copy(out=yT16[:, d4 * 4:(d4 + 1) * 4, ccsl],
                                           in_=pt[:])
            for rt in range(RT):
                nc.sync.dma_start(out=inters_o[rt * P:(rt + 1) * P, :],
                                  in_=idxt[rt, 0][:])
                nc.sync.dma_start(out=intras_o[rt * P:(rt + 1) * P, :],
                                  in_=idxt[rt, 1][:])
            tpp.release()
            gp.release()
            zp = tc.alloc_tile_pool(name="zpsum", bufs=1, space="PSUM")
            tpo = tc.alloc_tile_pool(name="tpopsum", bufs=2, space="PSUM")

            # 3a': h = hx + y @ w1y per half
            for half in range(2):
                csl = slice(half * R, (half + 1) * R)
                for h in range(KH):
                    hsl = slice(h * P, (h + 1) * P)
                    py = accp.tile([P, R], f32, tag="acc")
                    for k in range(KD):
                        w = wp.tile([P, P], bf16, tag="w1y")
                        nc.sync.dma_start(out=w[:],
                                          in_=w1y16[k * P:(k + 1) * P, hsl])
                        nc.tensor.matmul(out=py[:], lhsT=w[:],
                                         rhs=yT16[:, k, csl],
                                         start=(k == 0), stop=(k == KD - 1))
                    nc.vector.tensor_tensor(out=hsb[h][:, csl], in0=py[:],
                                            in1=hxsb[h][:], op=OP.add)

            # 3b+3c fused: per d: m, gates (1+sigmoid stored), and the
            # pair-A fc matmuls software-pipelined one d behind; pair B as a
            # dense tail loop. fc in [C, rows] layout, transposed back at end.
            MT = (slice(0, P), slice(P, C))
            for half in range(2):
                csl = slice(half * R, (half + 1) * R)
                gx1 = {}
                gy1 = {}
                ua = {}
                ub = {}
                zbA = {}
                for mt in range(2):
                    zbA[0, mt] = zp.tile([P if mt == 0 else C - P, R], f32,
                                         tag=f"zb0_{mt}", name=f"zbA0_{mt}")
                    zbA[1, mt] = zp.tile([P if mt == 0 else C - P, R], f32,
                                         tag=f"zb1_{mt}", name=f"zbA1_{mt}")

                def emit_fcA(d):
                    for mt in range(2):
                        wslice = wfct[d][:, MT[mt]]
                        nc.tensor.matmul(out=zbA[0, mt][:], lhsT=wslice,
                                         rhs=ua[d][:], start=(d == 0),
                                         stop=(d == KD - 1))
                        nc.tensor.matmul(out=zbA[1, mt][:], lhsT=wslice,
                                         rhs=ub[d][:], start=(d == 0),
                                         stop=(d == KD - 1))

                for d in range(KD):
                    pm = accp.tile([P, R], f32, tag="acc")
                    dsl = slice(d * P, (d + 1) * P)
                    for kh in range(KH):
                        nc.tensor.matmul(out=pm[:], lhsT=w2t[kh][:, dsl],
                                         rhs=hsb[kh][:, csl],
                                         start=(kh == 0), stop=False)
                    nc.tensor.matmul(out=pm[:], lhsT=b2t[:1, dsl],
                                     rhs=onest[:1, :R], start=False, stop=True)
                    msb = gw.tile([P, R], bf16, tag="msb")
                    nc.scalar.activation(msb[:], pm[:], AF.Copy)
                    p1 = gw.tile([P, R], bf16, tag="p1")
                    nc.vector.tensor_tensor(out=p1[:], in0=msb[:], in1=myhi[d][:],
                                            op=OP.mult)
                    p2 = gw.tile([P, R], bf16, tag="p2")
                    nc.vector.tensor_tensor(out=p2[:], in0=msb[:],
                                            in1=yT16[:, d, csl], op=OP.mult)
                    sx = gw.tile([P, R], bf16, tag="sx")
                    nc.scalar.activation(sx[:], p1[:], AF.Sigmoid)
                    sy = gw.tile([P, R], bf16, tag="sy")
                    nc.scalar.activation(sy[:], p2[:], AF.Sigmoid)
                    g1 = cp.tile([P, R], bf16, tag=f"gx1_{d}", name=f"gx1_{d}")
                    nc.vector.tensor_scalar(out=g1[:], in0=sx[:], scalar1=1.0,
                                            scalar2=None, op0=OP.add)
                    gx1[d] = g1
                    g2 = cp.tile([P, R], bf16, tag=f"gy1_{d}", name=f"gy1_{d}")
                    nc.vector.tensor_scalar(out=g2[:], in0=sy[:], scalar1=1.0,
                                            scalar2=None, op0=OP.add)
                    gy1[d] = g2
                    # pair-A products: x*(1+gx), x*(1+gy)
                    t = gw.tile([P, R], bf16, tag=f"ua{d % 3}", name=f"ua_{d}")
                    nc.vector.tensor_tensor(out=t[:], in0=myhi[d][:], in1=g1[:],
                                            op=OP.mult)
                    ua[d] = t
                    t = gw.tile([P, R], bf16, tag=f"ub{d % 3}", name=f"ub_{d}")
                    nc.vector.tensor_tensor(out=t[:], in0=myhi[d][:], in1=g2[:],
                                            op=OP.mult)
                    ub[d] = t
                    if d > 0:
                        emit_fcA(d - 1)
                emit_fcA(KD - 1)

                def epilogue(zb, j0):
                    for jj in range(2):
                        j = j0 + jj
                        zs = {}
                        for mt in range(2):
                            cn = P if mt == 0 else C - P
                            zt = g3.tile([cn, R], f32, tag=f"zs{mt}",
                                         name=f"zs{mt}")
                            nc.scalar.activation(zt[:], zb[jj, mt][:],
                                                 AF.Sigmoid,
                                                 bias=bfcct[mt][:, :1])
                            zs[mt] = zt
                        for rb in range(RT):
                            rbs = slice(rb * P, (rb + 1) * P)
                            po = tpo.tile([P, C], f32, tag="tpo")
                            nc.tensor.transpose(out=po[:, :P],
                                                in_=zs[0][:, rbs],
                                                identity=identft[:])
                            nc.tensor.transpose(out=po[:, P:C],
                                                in_=zs[1][:, rbs],
                                                identity=identft[:C - P, :C - P])
                            ot = g3.tile([P, C], f32, tag="ot")
                            nc.scalar.activation(ot[:], po[:], AF.Copy)
                            nc.sync.dma_start(
                                out=o_ext[j][half * R + rb * P:
                                             half * R + (rb + 1) * P, :],
                                in_=ot[:])

                epilogue(zbA, 0)

                # pair B: y*(1+gy) -> logit2_self, y*(1+gx) -> logit2_other
                zbB = {}
                for mt in range(2):
                    zbB[0, mt] = zp.tile([P if mt == 0 else C - P, R], f32,
                                         tag=f"zb0_{mt}", name=f"zbB0_{mt}")
                    zbB[1, mt] = zp.tile([P if mt == 0 else C - P, R], f32,
                                         tag=f"zb1_{mt}", name=f"zbB1_{mt}")
                for d in range(KD):
                    t1 = gw.tile([P, R], bf16, tag=f"ua{d % 3}", name=f"va_{d}")
                    nc.vector.tensor_tensor(out=t1[:], in0=yT16[:, d, csl],
                                            in1=gy1[d][:], op=OP.mult)
                    t2 = gw.tile([P, R], bf16, tag=f"ub{d % 3}", name=f"vb_{d}")
                    nc.vector.tensor_tensor(out=t2[:], in0=yT16[:, d, csl],
                                            in1=gx1[d][:], op=OP.mult)
                    for mt in range(2):
                        wslice = wfct[d][:, MT[mt]]
                        nc.tensor.matmul(out=zbB[0, mt][:], lhsT=wslice,
                                         rhs=t1[:], start=(d == 0),
                                         stop=(d == KD - 1))
                        nc.tensor.matmul(out=zbB[1, mt][:], lhsT=wslice,
                                         rhs=t2[:], start=(d == 0),
                                         stop=(d == KD - 1))
                epilogue(zbB, 2)

            tpo.release()
            zp.release()
            accp.release()
            gw.release()
            g3.release()
            wp.release()

    nc.finalize()
    return nc


def _get_built():
    global _BUILT
    if _BUILT is None:
        _BUILT = _build()
    return _BUILT


def kernel(**inputs):
    bf = ml_dtypes.bfloat16
    feats = np.asarray(inputs["feats"], dtype=np.float32)
    targets = np.asarray(inputs["targets"])
    w1 = np.asarray(inputs["w1"], dtype=np.float32)
    b1 = np.asarray(inputs["b1"], dtype=np.float32)
    w2 = np.asarray(inputs["w2"], dtype=np.float32)
    b2 = np.asarray(inputs["b2"], dtype=np.float32)
    wfc = np.asarray(inputs["wfc"], dtype=np.float32)
    bfc = np.asarray(inputs["bfc"], dtype=np.float32)

    f64 = feats.astype(np.float64)
    sqh = (0.5 * (f64 * f64).sum(axis=1)).astype(np.float32)
    featsT = np.ascontiguousarray(feats.T)                # (D, N) f32
    hiT = featsT.astype(bf)
    loT = (featsT - hiT.astype(np.float32)).astype(bf)
    feats16 = np.ascontiguousarray(hiT.T)                 # (N, D) bf16
    tf = targets.astype(np.float32)
    tall_b = np.ascontiguousarray(np.broadcast_to(tf.astype(bf), (P, N)))
    sqh_b = np.ascontiguousarray(np.broadcast_to(sqh, (P, N)))

    shared = dict(
        fallT_hi=np.ascontiguousarray(hiT),
        fallT_lo=np.ascontiguousarray(loT),
        feats16=feats16,
        sqh_b=sqh_b,
        tall_b=tall_b,
        w1x16=w1[:D].astype(bf),
        w1y16=w1[D:].astype(bf),
        w2_16=w2.astype(bf),
        wfc16=wfc.astype(bf),
        b1r=b1.reshape(1, H).astype(bf),
        b2r=b2.reshape(1, D).astype(bf),
        bfcr=bfc.reshape(1, C).astype(bf),
        onesr=np.ones((1, CW), dtype=bf),
        ident=np.eye(P, dtype=np.float32).astype(bf),
        identf=np.eye(P, dtype=np.float32),
        bfcc=bfc.reshape(C, 1).astype(np.float32),
    )
    in_maps = []
    for c in range(NCORES):
        rs = slice(c * R, (c + 1) * R)
        m = dict(shared)
        m["fmyT_hi"] = np.ascontiguousarray(hiT[:, rs])
        m["fmyT_lo"] = np.ascontiguousarray(loT[:, rs])
        m["tmy"] = np.ascontiguousarray(tf[rs].reshape(R, 1))
        in_maps.append(m)

    nc = _get_built()
    res = run_bass_kernel_spmd(nc, in_maps, core_ids=list(range(NCORES)),
                               trace=False)
    rs_ = res.results

    o = [np.empty((2 * N, C), dtype=np.float32) for _ in range(4)]
    inters_all = np.empty(N, dtype=np.int64)
    intras_all = np.empty(N, dtype=np.int64)
    for c in range(NCORES):
        for j in range(4):
            blk = rs_[c][f"o{j}"]
            o[j][c * R:(c + 1) * R] = blk[:R]
            o[j][N + c * R:N + (c + 1) * R] = blk[R:]
        inters_all[c * R:(c + 1) * R] = rs_[c]["inters_o"][:, 0]
        intras_all[c * R:(c + 1) * R] = rs_[c]["intras_o"][:, 0]

    kernel.last_idx = (intras_all, inters_all)
    labels1 = np.concatenate([targets, targets])
    labels2 = np.concatenate([targets, targets[inters_all]])
    return (o[0], o[1], o[2], o[3], labels1, labels2)


# revision 29
# speedup vs baseline: 1.9889x; 1.2340x over previous
"""APINet kernel for 8 Trainium2 NeuronCores.

Strategy (all compute on-device, no collectives needed):
  - Host replicates feats (as transposed bf16 hi/lo splits) to all 8 cores;
    each core owns 512 rows (data-parallel over N).
  - Phase 1 (per core): pairwise-distance scores for its 512 rows against
    all 4096 columns via bf16x3 matmuls (hi*hi + hi*lo + lo*hi accumulated
    in f32 PSUM, accuracy ~2.5e-4 on a min-gap of 4e-3), class masks as
    exact additive {0, -BIG} penalties, per-chunk top-2 scan with
    nc.vector.max/max_index, then a tiny cross-chunk argmax merge.
    Yields intra/inter nearest-neighbor indices identical to the f32
    reference argmin/argsort.
  - Phase 2: indirect-DMA row gathers of the neighbor features (bf16) and
    PE transposes into [D, rows] layout.
  - Phase 3: the APINet MLP entirely in [feature, row] layout (bf16
    matmuls, f32 PSUM): h = x@w1x + y@w1y, m = h@w2, sigmoid gates, and
    four fc heads with a shared x@wfc term; outputs sigmoid logits f32.
  - Host reassembles the (8192, 200) logits and integer labels.
"""

import numpy as np
import ml_dtypes

import concourse.bass as bass
import concourse.bacc as bacc
import concourse.mybir as mybir
import concourse.tile as tile
from concourse.bass_utils import run_bass_kernel_spmd

P = 128
N, D, H, C = 4096, 2048, 512, 200
NCORES = 8
R = N // NCORES          # 512 rows per core
RT = R // P              # 4 row-tiles
KD = D // P              # 16 k-tiles over D
KH = H // P              # 4 k-tiles over H
NCHUNK = 8               # column chunks of 512
CW = N // NCHUNK         # 512 chunk width
BIG = 1.0e6

f32 = mybir.dt.float32
bf16 = mybir.dt.bfloat16
i32 = mybir.dt.int32
u32 = mybir.dt.uint32
AX = mybir.AxisListType.X
OP = mybir.AluOpType
AF = mybir.ActivationFunctionType

_BUILT = None


def _phase1(nc, tc, ins, cp, bp):
    """Distance scores + masked per-chunk top-2 + cross-chunk merge.

    Returns (myhi tiles, idx tiles dict, const tiles dict).
    """
    (fallT_hi, fallT_lo, fmyT_hi, fmyT_lo, sqh_b, tall_b, tmy) = ins

    c1p = tc.alloc_tile_pool(name="p1const", bufs=1)
    myhi = [cp.tile([P, R], bf16, tag=f"myhi{k}", name=f"myhi{k}")
            for k in range(KD)]
    mylo = [c1p.tile([P, R], bf16, tag=f"mylo{k}", name=f"mylo{k}")
            for k in range(KD)]
    tallb = c1p.tile([P, N], bf16, tag="tallb")
    sqhb = c1p.tile([P, N], f32, tag="sqhb")
    tmyt = []
    for rt in range(RT):
        t = c1p.tile([P, 1], f32, tag=f"tmy{rt}")
        nc.sync.dma_start(out=t[:], in_=tmy[rt * P:(rt + 1) * P, :])
        tmyt.append(t)

    vbuf = {}
    ibuf = {}
    for rt in range(RT):
        for m in range(2):  # 0 = inter, 1 = intra
            vbuf[rt, m] = bp.tile([P, 2 * NCHUNK], f32, tag=f"v{rt}_{m}", name=f"v{rt}_{m}")
            ibuf[rt, m] = bp.tile([P, 2 * NCHUNK], f32, tag=f"i{rt}_{m}", name=f"i{rt}_{m}")

    sp = tc.alloc_tile_pool(name="p1stream", bufs=3)
    mp = tc.alloc_tile_pool(name="p1mask", bufs=3)
    pp1 = tc.alloc_tile_pool(name="p1psum", bufs=2, space="PSUM")
    for n in range(NCHUNK):
        nsl = slice(n * CW, (n + 1) * CW)
        Pt = [pp1.tile([P, CW], f32, tag=f"P{rt}", name=f"Pt{rt}") for rt in range(RT)]
        if n == 0:
            nc.sync.dma_start(out=sqhb[:, nsl], in_=sqh_b[:, nsl])
            nc.sync.dma_start(out=tallb[:, nsl], in_=tall_b[:, nsl])
        for k in range(KD):
            if n == 0:
                nc.sync.dma_start(out=myhi[k][:],
                                  in_=fmyT_hi[k * P:(k + 1) * P, :])
                nc.sync.dma_start(out=mylo[k][:],
                                  in_=fmyT_lo[k * P:(k + 1) * P, :])
            elif k == 0:
                nc.sync.dma_start(out=sqhb[:, nsl], in_=sqh_b[:, nsl])
                nc.sync.dma_start(out=tallb[:, nsl], in_=tall_b[:, nsl])
            ahi = sp.tile([P, CW], bf16, tag="ahi")
            nc.sync.dma_start(out=ahi[:], in_=fallT_hi[k * P:(k + 1) * P, nsl])
            alo = sp.tile([P, CW], bf16, tag="alo")
            nc.sync.dma_start(out=alo[:], in_=fallT_lo[k * P:(k + 1) * P, nsl])
            for rt in range(RT):
                rsl = slice(rt * P, (rt + 1) * P)
                nc.tensor.matmul(out=Pt[rt][:], lhsT=myhi[k][:, rsl],
                                 rhs=ahi[:], start=(k == 0), stop=False)
                nc.tensor.matmul(out=Pt[rt][:], lhsT=myhi[k][:, rsl],
                                 rhs=alo[:], start=False, stop=False)
                nc.tensor.matmul(out=Pt[rt][:], lhsT=mylo[k][:, rsl],
                                 rhs=ahi[:], start=False, stop=(k == KD - 1))
        for rt in range(RT):
            baset = mp.tile([P, CW], f32, tag="base")
            nc.vector.tensor_tensor(out=baset[:], in0=Pt[rt][:],
                                    in1=sqhb[:, nsl], op=OP.subtract)
            q1 = mp.tile([P, CW], f32, tag="q1")
            nc.vector.tensor_scalar(out=q1[:], in0=tallb[:, nsl],
                                    scalar1=tmyt[rt][:, :1], scalar2=BIG,
                                    op0=OP.is_equal, op1=OP.mult)
            q2 = mp.tile([P, CW], f32, tag="q2")
            nc.vector.tensor_scalar(out=q2[:], in0=tallb[:, nsl],
                                    scalar1=tmyt[rt][:, :1], scalar2=BIG,
                                    op0=OP.not_equal, op1=OP.mult)
            for m, q in ((0, q1), (1, q2)):
                mt = mp.tile([P, CW], f32, tag=f"mt{m}")
                nc.vector.tensor_tensor(out=mt[:], in0=baset[:], in1=q[:],
                                        op=OP.subtract)
                v8 = mp.tile([P, 8], f32, tag="v8")
                nc.vector.max(out=v8[:], in_=mt[:])
                i8 = mp.tile([P, 8], u32, tag="i8")
                nc.vector.max_index(out=i8[:], in_max=v8[:], in_values=mt[:])
                nc.vector.tensor_copy(out=vbuf[rt, m][:, 2 * n:2 * n + 2],
                                      in_=v8[:, :2])
                i2f = mp.tile([P, 2], f32, tag="i2f")
                nc.vector.tensor_copy(out=i2f[:], in_=i8[:, :2])
                nc.vector.tensor_scalar(out=ibuf[rt, m][:, 2 * n:2 * n + 2],
                                        in0=i2f[:], scalar1=float(n * CW),
                                        scalar2=None, op0=OP.add)
    pp1.release()
    mp.release()
    sp.release()
    c1p.release()

    return myhi, vbuf, ibuf


def _merge_one(nc, bp, vbuf, ibuf, rt, m):
    """Cross-chunk merge -> final index tile for one (rt, mask)."""
    if True:
        if True:
            g8 = bp.tile([P, 8], f32, tag=f"g8_{rt}_{m}", name=f"g8_{rt}_{m}")
            nc.vector.max(out=g8[:], in_=vbuf[rt, m][:])
            # inter wants rank 0; intra wants rank 1 (rank 0 is self)
            v = g8[:, m:m + 1]
            eq = bp.tile([P, 2 * NCHUNK], f32, tag=f"eq_{rt}_{m}", name=f"eq_{rt}_{m}")
            nc.vector.tensor_scalar(out=eq[:], in0=vbuf[rt, m][:], scalar1=v,
                                    scalar2=None, op0=OP.is_equal)
            t1 = bp.tile([P, 2 * NCHUNK], f32, tag=f"t1_{rt}_{m}", name=f"t1_{rt}_{m}")
            nc.vector.tensor_tensor(out=t1[:], in0=ibuf[rt, m][:], in1=eq[:],
                                    op=OP.mult)
            t2 = bp.tile([P, 2 * NCHUNK], f32, tag=f"t2_{rt}_{m}", name=f"t2_{rt}_{m}")
            nc.vector.tensor_scalar(out=t2[:], in0=eq[:], scalar1=-1.0e9,
                                    scalar2=1.0e9, op0=OP.mult, op1=OP.add)
            cand = bp.tile([P, 2 * NCHUNK], f32, tag=f"cand_{rt}_{m}", name=f"cand_{rt}_{m}")
            nc.vector.tensor_tensor(out=cand[:], in0=t1[:], in1=t2[:], op=OP.add)
            idxf = bp.tile([P, 1], f32, tag=f"idxf_{rt}_{m}", name=f"idxf_{rt}_{m}")
            nc.vector.tensor_reduce(idxf[:], cand[:], AX, OP.min)
            ii = bp.tile([P, 1], i32, tag=f"idxi_{rt}_{m}", name=f"idxi_{rt}_{m}")
            nc.vector.tensor_copy(out=ii[:], in_=idxf[:])
    return ii


def _build():
    nc = bacc.Bacc(None, target_bir_lowering=False, debug=False)
    dp = nc.declare_dram_parameter

    fallT_hi = dp("fallT_hi", [D, N], bf16, isOutput=False)
    fallT_lo = dp("fallT_lo", [D, N], bf16, isOutput=False)
    fmyT_hi = dp("fmyT_hi", [D, R], bf16, isOutput=False)
    fmyT_lo = dp("fmyT_lo", [D, R], bf16, isOutput=False)
    feats16 = dp("feats16", [N, D], bf16, isOutput=False)
    sqh_b = dp("sqh_b", [P, N], f32, isOutput=False)
    tall_b = dp("tall_b", [P, N], bf16, isOutput=False)
    tmy = dp("tmy", [R, 1], f32, isOutput=False)
    w1x16 = dp("w1x16", [D, H], bf16, isOutput=False)
    w1y16 = dp("w1y16", [D, H], bf16, isOutput=False)
    w2_16 = dp("w2_16", [H, D], bf16, isOutput=False)
    wfc16 = dp("wfc16", [D, C], bf16, isOutput=False)
    b1r = dp("b1r", [1, H], bf16, isOutput=False)
    b2r = dp("b2r", [1, D], bf16, isOutput=False)
    bfcr = dp("bfcr", [1, C], bf16, isOutput=False)
    onesr = dp("onesr", [1, CW], bf16, isOutput=False)
    ident = dp("ident", [P, P], bf16, isOutput=False)
    identf = dp("identf", [P, P], f32, isOutput=False)
    bfcc = dp("bfcc", [C, 1], f32, isOutput=False)

    o_ext = [dp(f"o{j}", [2 * R, C], f32, isOutput=True) for j in range(4)]
    inters_o = dp("inters_o", [R, 1], i32, isOutput=True)
    intras_o = dp("intras_o", [R, 1], i32, isOutput=True)

    with tile.TileContext(nc) as tc:
        with (
            tc.tile_pool(name="const", bufs=1) as cp,
            tc.tile_pool(name="p1buf", bufs=1) as bp,
        ):
            # ---- phase 1: distance scores + per-chunk top-2 scans ----
            myhi, vbuf, ibuf = _phase1(
                nc, tc,
                (fallT_hi, fallT_lo, fmyT_hi, fmyT_lo, sqh_b, tall_b, tmy),
                cp, bp)

            # phase-3 pools that we want alive early (hx/zx0 fill the
            # scan->gather latency gap with y-independent matmuls)
            wp = tc.alloc_tile_pool(name="wstream", bufs=3)
            g3 = tc.alloc_tile_pool(name="p3work", bufs=2)
            gw = tc.alloc_tile_pool(name="gatework", bufs=3)
            up = tc.alloc_tile_pool(name="p3u", bufs=1)
            accp = tc.alloc_tile_pool(name="accpsum", bufs=3, space="PSUM")
            z0p = tc.alloc_tile_pool(name="z0psum", bufs=1, space="PSUM")

            onest = cp.tile([1, CW], bf16, tag="ones")
            nc.sync.dma_start(out=onest[:], in_=onesr[:])
            b1t = cp.tile([1, H], bf16, tag="b1")
            nc.sync.dma_start(out=b1t[:], in_=b1r[:])
            wfct = []
            for k in range(KD):
                t = cp.tile([P, C], bf16, tag=f"wfc{k}")
                nc.sync.dma_start(out=t[:], in_=wfc16[k * P:(k + 1) * P, :])
                wfct.append(t)

            # 3a: hx = x @ w1x + b1 (shared by both halves)
            hxsb = []
            for h in range(KH):
                hsl = slice(h * P, (h + 1) * P)
                px = accp.tile([P, R], f32, tag="acc")
                for k in range(KD):
                    w = wp.tile([P, P], bf16, tag="w1x")
                    nc.sync.dma_start(out=w[:], in_=w1x16[k * P:(k + 1) * P, hsl])
                    nc.tensor.matmul(out=px[:], lhsT=w[:], rhs=myhi[k][:],
                                     start=(k == 0), stop=False)
                nc.tensor.matmul(out=px[:], lhsT=b1t[:1, hsl], rhs=onest[:1, :R],
                                 start=False, stop=True)
                hx = cp.tile([P, R], bf16, tag=f"hx{h}")
                nc.scalar.activation(hx[:], px[:], AF.Copy)
                hxsb.append(hx)

            # 3c-x: zx0 = x @ wfc + bfc (shared by both halves)
            bfct = cp.tile([1, C], bf16, tag="bfc")
            nc.sync.dma_start(out=bfct[:], in_=bfcr[:])
            zx0sb = [cp.tile([P, C], f32, tag=f"zx0_{ms}", name=f"zx0_{ms}")
                     for ms in range(RT)]
            for ms in range(RT):
                msl = slice(ms * P, (ms + 1) * P)
                pz = z0p.tile([P, C], f32, tag="z0")
                for k in range(KD):
                    nc.tensor.matmul(out=pz[:], lhsT=myhi[k][:, msl],
                                     rhs=wfct[k][:], start=(k == 0), stop=False)
                nc.tensor.matmul(out=pz[:], lhsT=onest[:1, :P], rhs=bfct[:1, :],
                                 start=False, stop=True)
                nc.scalar.activation(zx0sb[ms][:], pz[:], AF.Copy)

            # ---- phase 2: gather neighbor rows (bf16) + transpose ----
            identt = cp.tile([P, P], bf16, tag="ident")
            nc.sync.dma_start(out=identt[:], in_=ident[:])
            yT16 = cp.tile([P, KD, 2 * R], bf16, tag="yT16")
            gp = tc.alloc_tile_pool(name="gpool", bufs=3)
            tpp = tc.alloc_tile_pool(name="tpsum", bufs=2, space="PSUM")

            # ---- phase 3 (rest): needs y ----
            b2t = cp.tile([1, D], bf16, tag="b2")
            nc.sync.dma_start(out=b2t[:], in_=b2r[:])
            w2t = []
            for kh in range(KH):
                t = cp.tile([P, D], bf16, tag=f"w2_{kh}")
                nc.sync.dma_start(out=t[:], in_=w2_16[kh * P:(kh + 1) * P, :])
                w2t.append(t)
            hsb = [cp.tile([P, 2 * R], bf16, tag=f"h{kh}", name=f"hsb{kh}")
                   for kh in range(KH)]

            # phase 1b + 2: merge -> gather -> transpose per (rt, m)
            idxt = {}
            for half, m in ((0, 1), (1, 0)):  # half 0 = intra, 1 = inter
                for rt in range(RT):
                    idxt[rt, m] = _merge_one(nc, bp, vbuf, ibuf, rt, m)
                    yg = gp.tile([P, D], bf16, tag="yg")
                    nc.gpsimd.indirect_dma_start(
                        out=yg[:], out_offset=None, in_=feats16[:],
                        in_offset=bass.IndirectOffsetOnAxis(
                            ap=idxt[rt, m][:, :1], axis=0))
                    for d4 in range(KD // 4):
                        pt = tpp.tile([P, 4, P], bf16, tag="tp")
                        for i in range(4):
                            d = d4 * 4 + i
                            nc.tensor.transpose(out=pt[:, i, :],
                                                in_=yg[:, d * P:(d + 1) * P],
                                                identity=identt[:])
                        ccsl = slice(half * R + rt * P, half * R + (rt + 1) * P)
                        nc.any.tensor_copy(out=yT16[:, d4 * 4:(d4 + 1) * 4, ccsl],
                                           in_=pt[:])
            for rt in range(RT):
                nc.sync.dma_start(out=inters_o[rt * P:(rt + 1) * P, :],
                                  in_=idxt[rt, 0][:])
                nc.sync.dma_start(out=intras_o[rt * P:(rt + 1) * P, :],
                                  in_=idxt[rt, 1][:])
            tpp.release()
            gp.release()
            fcp = tc.alloc_tile_pool(name="fcpsum", bufs=1, space="PSUM")

            # 3a': h = hx + y @ w1y per half
            for half in range(2):
                csl = slice(half * R, (half + 1) * R)
                for h in range(KH):
                    hsl = slice(h * P, (h + 1) * P)
                    py = accp.tile([P, R], f32, tag="acc")
                    for k in range(KD):
                        w = wp.tile([P, P], bf16, tag="w1y")
                        nc.sync.dma_start(out=w[:],
                                          in_=w1y16[k * P:(k + 1) * P, hsl])
                        nc.tensor.matmul(out=py[:], lhsT=w[:],
                                         rhs=yT16[:, k, csl],
                                         start=(k == 0), stop=(k == KD - 1))
                    nc.vector.tensor_tensor(out=hsb[h][:, csl], in0=py[:],
                                            in1=hxsb[h][:], op=OP.add)

            # 3b+3c per half: m, gates, products, fc
            for half in range(2):
                csl = slice(half * R, (half + 1) * R)
                u = {}
                for d in range(KD):
                    pm = accp.tile([P, R], f32, tag="acc")
                    dsl = slice(d * P, (d + 1) * P)
                    for kh in range(KH):
                        nc.tensor.matmul(out=pm[:], lhsT=w2t[kh][:, dsl],
                                         rhs=hsb[kh][:, csl],
                                         start=(kh == 0), stop=False)
                    nc.tensor.matmul(out=pm[:], lhsT=b2t[:1, dsl],
                                     rhs=onest[:1, :R], start=False, stop=True)
                    msb = gw.tile([P, R], bf16, tag="msb")
                    nc.scalar.activation(msb[:], pm[:], AF.Copy)
                    p1 = gw.tile([P, R], bf16, tag="p1")
                    nc.vector.tensor_tensor(out=p1[:], in0=msb[:], in1=myhi[d][:],
                                            op=OP.mult)
                    p2 = gw.tile([P, R], bf16, tag="p2")
                    nc.vector.tensor_tensor(out=p2[:], in0=msb[:],
                                            in1=yT16[:, d, csl], op=OP.mult)
                    gx = gw.tile([P, R], bf16, tag="gx")
                    nc.scalar.activation(gx[:], p1[:], AF.Sigmoid)
                    gy = gw.tile([P, R], bf16, tag="gy")
                    nc.scalar.activation(gy[:], p2[:], AF.Sigmoid)
                    for j, (xa, g) in enumerate(
                            ((True, gx), (True, gy), (False, gy), (False, gx))):
                        ut = up.tile([P, R], bf16, tag=f"u{j}_{d}", name=f"u{j}_{d}")
                        a_ap = myhi[d][:] if xa else yT16[:, d, csl]
                        nc.vector.tensor_tensor(out=ut[:], in0=a_ap, in1=g[:],
                                                op=OP.mult)
                        u[j, d] = ut

                # fc for this half
                for ms in range(RT):
                    msl = slice(ms * P, (ms + 1) * P)
                    pzy = z0p.tile([P, C], f32, tag="z0")
                    for k in range(KD):
                        nc.tensor.matmul(out=pzy[:],
                                         lhsT=yT16[:, k, half * R + ms * P:
                                                   half * R + (ms + 1) * P],
                                         rhs=wfct[k][:], start=(k == 0),
                                         stop=False)
                    nc.tensor.matmul(out=pzy[:], lhsT=onest[:1, :P],
                                     rhs=bfct[:1, :], start=False, stop=True)
                    zy0 = g3.tile([P, C], f32, tag="zy0")
                    nc.scalar.activation(zy0[:], pzy[:], AF.Copy)
                    zp_ = []
                    for j in range(4):
                        pzj = fcp.tile([P, C], f32, tag=f"z{j + 1}", name=f"z{j+1}")
                        for k in range(KD):
                            nc.tensor.matmul(out=pzj[:], lhsT=u[j, k][:, msl],
                                             rhs=wfct[k][:], start=(k == 0),
                                             stop=(k == KD - 1))
                        zp_.append(pzj)
                    for j in range(4):
                        z0 = zx0sb[ms] if j < 2 else zy0
                        s = g3.tile([P, C], f32, tag=f"s{j}", name=f"s{j}")
                        nc.vector.tensor_tensor(out=s[:], in0=zp_[j][:],
                                                in1=z0[:], op=OP.add)
                        o = g3.tile([P, C], f32, tag=f"o{j}", name=f"o{j}")
                        nc.scalar.activation(o[:], s[:], AF.Sigmoid)
                        nc.sync.dma_start(
                            out=o_ext[j][half * R + ms * P:
                                         half * R + (ms + 1) * P, :],
                            in_=o[:])
            fcp.release()
            z0p.release()
            accp.release()
            up.release()
            gw.release()
            g3.release()
            wp.release()

    nc.finalize()
    return nc


def _get_built():
    global _BUILT
    if _BUILT is None:
        _BUILT = _build()
    return _BUILT


def kernel(**inputs):
    bf = ml_dtypes.bfloat16
    feats = np.asarray(inputs["feats"], dtype=np.float32)
    targets = np.asarray(inputs["targets"])
    w1 = np.asarray(inputs["w1"], dtype=np.float32)
    b1 = np.asarray(inputs["b1"], dtype=np.float32)
    w2 = np.asarray(inputs["w2"], dtype=np.float32)
    b2 = np.asarray(inputs["b2"], dtype=np.float32)
    wfc = np.asarray(inputs["wfc"], dtype=np.float32)
    bfc = np.asarray(inputs["bfc"], dtype=np.float32)

    f64 = feats.astype(np.float64)
    sqh = (0.5 * (f64 * f64).sum(axis=1)).astype(np.float32)
    featsT = np.ascontiguousarray(feats.T)                # (D, N) f32
    hiT = featsT.astype(bf)
    loT = (featsT - hiT.astype(np.float32)).astype(bf)
    feats16 = np.ascontiguousarray(hiT.T)                 # (N, D) bf16
    tf = targets.astype(np.float32)
    tall_b = np.ascontiguousarray(np.broadcast_to(tf.astype(bf), (P, N)))
    sqh_b = np.ascontiguousarray(np.broadcast_to(sqh, (P, N)))

    shared = dict(
        fallT_hi=np.ascontiguousarray(hiT),
        fallT_lo=np.ascontiguousarray(loT),
        feats16=feats16,
        sqh_b=sqh_b,
        tall_b=tall_b,
        w1x16=w1[:D].astype(bf),
        w1y16=w1[D:].astype(bf),
        w2_16=w2.astype(bf),
        wfc16=wfc.astype(bf),
        b1r=b1.reshape(1, H).astype(bf),
        b2r=b2.reshape(1, D).astype(bf),
        bfcr=bfc.reshape(1, C).astype(bf),
        onesr=np.ones((1, CW), dtype=bf),
        ident=np.eye(P, dtype=np.float32).astype(bf),
        identf=np.eye(P, dtype=np.float32),
        bfcc=bfc.reshape(C, 1).astype(np.float32),
    )
    in_maps = []
    for c in range(NCORES):
        rs = slice(c * R, (c + 1) * R)
        m = dict(shared)
        m["fmyT_hi"] = np.ascontiguousarray(hiT[:, rs])
        m["fmyT_lo"] = np.ascontiguousarray(loT[:, rs])
        m["tmy"] = np.ascontiguousarray(tf[rs].reshape(R, 1))
        in_maps.append(m)

    nc = _get_built()
    res = run_bass_kernel_spmd(nc, in_maps, core_ids=list(range(NCORES)),
                               trace=False)
    rs_ = res.results

    o = [np.empty((2 * N, C), dtype=np.float32) for _ in range(4)]
    inters_all = np.empty(N, dtype=np.int64)
    intras_all = np.empty(N, dtype=np.int64)
    for c in range(NCORES):
        for j in range(4):
            blk = rs_[c][f"o{j}"]
            o[j][c * R:(c + 1) * R] = blk[:R]
            o[j][N + c * R:N + (c + 1) * R] = blk[R:]
        inters_all[c * R:(c + 1) * R] = rs_[c]["inters_o"][:, 0]
        intras_all[c * R:(c + 1) * R] = rs_[c]["intras_o"][:, 0]

    kernel.last_idx = (intras_all, inters_all)
    labels1 = np.concatenate([targets, targets])
    labels2 = np.concatenate([targets, targets[inters_all]])
    return (o[0], o[1], o[2], o[3], labels1, labels2)
